# revision 15
# baseline (speedup 1.0000x reference)
"""Trainium2 Bass kernel for nn_CoefficientDecoder.

reference computation (all f32):
    h = relu(x @ W1.T + b1)         x:[B,256] -> h:[B,64]
    h = h @ Wd3.T + bd3             [B,64]
    h = h @ Wd2.T + bd2             [B,64]
    h = h @ Wd1.T + bd1             [B,64]
    z = h @ W2.T + b2               [B,512]
    out = z @ bases                 bases:[512,4096] -> out:[B,4096]

Strategy: pure data-parallel over the batch dim across 8 NeuronCores
(B=8192 -> 1024 rows/core).

Everything after the ReLU is linear, so it is folded host-side:
    W2eff = W2@Wd1@Wd2@Wd3                     [512, 64]
    b2eff = b2 + (bd3@Wd2.T@Wd1.T + bd2@Wd1.T + bd1)@W2.T
    Beff  = W2eff.T @ bases                    [64, 4096]
    brow  = b2eff @ bases                      [4096]
    out   = relu(x@W1.T + b1) @ Beff + brow

The bias row rides along as contraction row 64: W1 gets a 65th output
channel with zero weights and bias 1 (relu(1)=1), and Beff gets brow as
row 64.  This collapses the big GEMM's contraction from K=512 to K=65,
cutting PE work 4x — which shifts the kernel from compute-bound to
DMA-bound (out is 16 MB/core in f32), so the whole pipeline runs in
fp16: x, W1, Beff loads and the out store (8 MB/core).  All fp16 keeps
rel err ~6e-4 (gate 2e-2); the DMA floor drops from ~25 MB to ~9.4 MB
per core.

Per-core schedule (all DMAs on the SP queue, in dependency order):
    load  xT j0 | beff s0-1 | xT j1 | beff s2-7      (f16, 4 DMAs + consts)
    L1    hT[65, j*512:+512] = relu(W1aug @ xT_j + b1aug)   2 j-chunks
    GEMM  mm-outer (stationary hT block reused 8x), s-inner:
          psum[128,512] = hT[:,mm*128:+128].T @ beff[:,s*512:+512]
          PSUM->SBUF f16 copies round-robin ACT/DVE/Pool
          stores per mm: [128,0:1024] after s1 (early), [128,1024:4096]
          after s7 — 16 big stores total, each 128 contiguous rows.
"""

import numpy as np

import concourse.bass as bass
import concourse.tile as tile
from concourse import bacc, mybir
from concourse.bass import ts
from concourse.bass_utils import run_bass_kernel_spmd

N_CORES = 8
B, IN_F, HID, NB, SEQ = 8192, 256, 64, 512, 4096
B_LOC = B // N_CORES            # 1024 batch rows per core
HA = HID + 1                    # 65: hidden + ones row (bias via matmul)

F32 = mybir.dt.float32
F16 = mybir.dt.float16

# kept for test.py compat; this kernel is fp16-only
GEMM_MODE = "f16"
OUT_MODE = "f16"

_CACHE = {}


def _build(gemm_mode: str = GEMM_MODE, out_mode: str = OUT_MODE, repeat: int = 1):
    out_dt = F32 if out_mode == "f32" else F16
    nc = bacc.Bacc(
        "TRN2",
        target_bir_lowering=False,
        debug=False,
        enable_asserts=False,
        num_devices=N_CORES,
    )

    xT_d = nc.declare_dram_parameter("xT", [IN_F, B_LOC], F16, isOutput=False)
    w1c_d = nc.declare_dram_parameter("w1c", [128, 2 * HA], F16, isOutput=False)
    b1c_d = nc.declare_dram_parameter("b1c", [128, 1], F32, isOutput=False)
    beff_d = nc.declare_dram_parameter("beff", [HA, SEQ], F16, isOutput=False)
    out_d = nc.declare_dram_parameter("out", [B_LOC, SEQ], out_dt, isOutput=True)

    NJ = 2                  # L1 batch chunks of 512
    MM = B_LOC // 128       # 8 batch blocks for the GEMM
    SC = SEQ // 512         # 8 seq chunks

    relu = mybir.ActivationFunctionType.Relu
    copyf = mybir.ActivationFunctionType.Copy

    with tile.TileContext(nc) as tc:
        with (
            tc.tile_pool(name="const", bufs=1) as constp,
            tc.tile_pool(name="data", bufs=1) as datap,
            tc.tile_pool(name="outsb", bufs=3) as outsbp,
            tc.tile_pool(name="h_ps", bufs=2, space="PSUM") as hpp,
            tc.tile_pool(name="o_ps", bufs=3, space="PSUM") as opp,
        ):
            def body():
                w1 = constp.tile([128, 2, HA], F16, tag="w1")
                b1sb = constp.tile([128, 1], F32, tag="b1")
                xsb = datap.tile([128, 2, B_LOC], F16, tag="x")
                beff = datap.tile([HA, SEQ], F16, tag="beff")
                hT = datap.tile([HA, B_LOC], F16, tag="hT")

                xT_pkn = xT_d.rearrange("(k p) n -> p k n", p=128)
                w1_pk = w1c_d.rearrange("p (k m) -> p k m", k=2)

                # consts on the ACT queue (no deps, issued while ACT is
                # idle); x + beff on SP in consumption order.  Data loads
                # must NOT share the ACT queue: a beff load waits on the
                # prior iteration's GEMM reads, and the in-order queue would
                # stall every drain queued behind it.
                nc.scalar.dma_start(w1[:], w1_pk[:])
                nc.scalar.dma_start(b1sb[:], b1c_d[:])
                nc.sync.dma_start(xsb[:, :, 0:512], xT_pkn[:, :, 0:512])
                nc.sync.dma_start(beff[:, 0:1024], beff_d[:, 0:1024])
                nc.sync.dma_start(xsb[:, :, 512:1024], xT_pkn[:, :, 512:1024])
                nc.sync.dma_start(beff[:, 1024:4096], beff_d[:, 1024:4096])

                # L1: hT[65, 1024] = relu(W1aug @ xT + b1aug), row 64 == 1.0
                for j in range(NJ):
                    hp = hpp.tile([HA, 512], F32, tag="h")
                    for k in range(2):
                        nc.tensor.matmul(
                            hp[:],
                            w1[:, k, :],
                            xsb[:, k, ts(j, 512)],
                            start=(k == 0),
                            stop=(k == 1),
                        )
                    nc.scalar.activation(
                        hT[:, ts(j, 512)], hp[:], relu, bias=b1sb[:HA, :]
                    )

                # GEMM: out[mm*128:+128, :] = hT[:, mm-block].T @ beff
                # PSUM drains in 2-bank [128, 1024] chunks, ACT/DVE weighted
                # ~17:15 (ACT is 1.2 GHz vs DVE 0.96, but also does the relus)
                NH = SEQ // 1024        # 4 drain chunks per mm block
                drain_ctr = 0
                for mm in range(MM):
                    osb = outsbp.tile([128, SEQ], out_dt, tag="osb")
                    for sh in range(NH):
                        op = opp.tile([128, 1024], F32, tag="op")
                        for t in range(2):
                            nc.tensor.matmul(
                                op[:, ts(t, 512)],
                                hT[:, ts(mm, 128)],
                                beff[:, ts(2 * sh + t, 512)],
                                start=True,
                                stop=True,
                            )
                        use_act = (drain_ctr * 17) // 32 != ((drain_ctr + 1) * 17) // 32
                        drain_ctr += 1
                        if use_act:
                            nc.scalar.activation(osb[:, ts(sh, 1024)], op[:], copyf)
                        else:
                            nc.vector.tensor_copy(osb[:, ts(sh, 1024)], op[:])
                        if sh == 0 and mm % 2 == 0:
                            # early partial store primes the pipe (SP mms
                            # only: SWDGE's ~1us per-DMA gen cost on the
                            # Pool sequencer makes extra gpsimd DMAs dear)
                            nc.sync.dma_start(
                                out_d[ts(mm, 128), 0:1024], osb[:, 0:1024]
                            )
                    if mm % 2 == 0:
                        nc.sync.dma_start(
                            out_d[ts(mm, 128), 1024:4096], osb[:, 1024:4096]
                        )
                    else:
                        nc.gpsimd.dma_start(out_d[ts(mm, 128), :], osb[:])

            if repeat == 1:
                body()
            else:
                with tc.For_i(0, repeat, 1):
                    body()

    nc.compile()
    return nc


def _get_nc(gemm_mode: str = GEMM_MODE, out_mode: str = OUT_MODE, repeat: int = 1):
    key = (gemm_mode, out_mode, repeat)
    if key not in _CACHE:
        _CACHE[key] = _build(gemm_mode, out_mode, repeat)
    return _CACHE[key]


def _fold(W1, b1, Wd1, bd1, Wd2, bd2, Wd3, bd3, W2, b2, bases):
    f8 = np.float64
    W2eff = W2.astype(f8) @ Wd1.astype(f8) @ Wd2.astype(f8) @ Wd3.astype(f8)
    b2eff = b2.astype(f8) + (
        bd3.astype(f8) @ Wd2.astype(f8).T @ Wd1.astype(f8).T
        + bd2.astype(f8) @ Wd1.astype(f8).T
        + bd1.astype(f8)
    ) @ W2.astype(f8).T
    beff = np.empty((HA, SEQ), np.float16)
    beff[:HID] = (W2eff.T @ bases.astype(f8)).astype(np.float16)
    beff[HID] = (b2eff @ bases.astype(f8)).astype(np.float16)

    # W1aug: 65th output channel with zero weights + bias 1 -> relu==1.0
    w1c = np.zeros((128, 2 * HA), np.float16)
    W1T = W1.T.astype(np.float16)          # [256, 64]
    w1c[:, 0:HID] = W1T[:128]
    w1c[:, HA : HA + HID] = W1T[128:]
    b1c = np.zeros((128, 1), np.float32)
    b1c[:HID, 0] = b1
    b1c[HID, 0] = 1.0
    return w1c, b1c, beff


def _in_maps(x, W1, b1, Wd1, bd1, Wd2, bd2, Wd3, bd3, W2, b2, bases,
             gemm_mode=GEMM_MODE):
    w1c, b1c, beff = _fold(W1, b1, Wd1, bd1, Wd2, bd2, Wd3, bd3, W2, b2, bases)
    common = {"w1c": w1c, "b1c": b1c, "beff": beff}
    maps = []
    for i in range(N_CORES):
        m = dict(common)
        m["xT"] = np.ascontiguousarray(
            x[i * B_LOC : (i + 1) * B_LOC].T.astype(np.float16)
        )
        maps.append(m)
    return maps


def run(inputs: dict, gemm_mode: str = GEMM_MODE, out_mode: str = OUT_MODE,
        repeat: int = 1, **run_kwargs):
    """Shard, execute on 8 cores, gather. Returns (out, BassKernelResults)."""
    nc = _get_nc(gemm_mode, out_mode, repeat)
    in_maps = _in_maps(**{k: np.asarray(v) for k, v in inputs.items()},
                       gemm_mode=gemm_mode)
    res = run_bass_kernel_spmd(nc, in_maps, list(range(N_CORES)), **run_kwargs)
    shards = [np.asarray(res.results[i]["out"], dtype=np.float32)
              for i in range(N_CORES)]
    out = np.concatenate(shards, axis=0)
    return out, res


def kernel(**inputs) -> np.ndarray:
    out, _ = run(inputs)
    return out


# revision 18
# speedup vs baseline: 1.3685x; 1.3685x over previous
"""Trainium2 Bass kernel for nn_CoefficientDecoder.

reference computation (all f32):
    h = relu(x @ W1.T + b1)         x:[B,256] -> h:[B,64]
    h = h @ Wd3.T + bd3             [B,64]
    h = h @ Wd2.T + bd2             [B,64]
    h = h @ Wd1.T + bd1             [B,64]
    z = h @ W2.T + b2               [B,512]
    out = z @ bases                 bases:[512,4096] -> out:[B,4096]

Strategy: pure data-parallel over the batch dim across 8 NeuronCores
(B=8192 -> 1024 rows/core).

Everything after the ReLU is linear, so it is folded host-side:
    W2eff = W2@Wd1@Wd2@Wd3                     [512, 64]
    b2eff = b2 + (bd3@Wd2.T@Wd1.T + bd2@Wd1.T + bd1)@W2.T
    Beff  = W2eff.T @ bases                    [64, 4096]
    brow  = b2eff @ bases                      [4096]
    out   = relu(x@W1.T + b1) @ Beff + brow

The bias row rides along as contraction row 64: W1 gets a 65th output
channel with zero weights and bias 1 (relu(1)=1), and Beff gets brow as
row 64.  This collapses the big GEMM's contraction from K=512 to K=65,
cutting PE work 4x — which shifts the kernel from compute-bound to
DMA-bound (out is 16 MB/core in f32), so the whole pipeline runs in
fp16: x, W1, Beff loads and the out store (8 MB/core).  All fp16 keeps
rel err ~6e-4 (gate 2e-2); the DMA floor drops from ~25 MB to ~9.4 MB
per core.

Per-core schedule:
    load  consts on the ACT queue; xT j0 | beff s0-1 | xT j1 | beff s2-7
          on SP in consumption order (data loads must not share a queue
          with instructions that wait on prior-iteration reads: in-order
          queues head-of-line block)
    L1    hT[65, j*512:+512] = relu(W1aug @ xT_j + b1aug)   2 j-chunks
    GEMM  mm-outer (stationary hT block reused 8x), s-inner:
          psum[128, sh*512+t] = hT[:,mm*128:+128].T @ beff[:,s*512:+512]
          PSUM drains in 2-bank [128,1024] chunks, weighted ACT/DVE
          stores: one big DMA per mm block (128 contiguous 8KB rows),
          round-robin across the SP / gpsimd-SWDGE / ACT queues (per-queue
          DGE issue cost is the constraint, so few + huge transfers win)
"""

import numpy as np

import concourse.bass as bass
import concourse.tile as tile
from concourse import bacc, mybir
from concourse.bass import ts
from concourse.bass_utils import run_bass_kernel_spmd

N_CORES = 8
B, IN_F, HID, NB, SEQ = 8192, 256, 64, 512, 4096
B_LOC = B // N_CORES            # 1024 batch rows per core
HA = HID + 1                    # 65: hidden + ones row (bias via matmul)

F32 = mybir.dt.float32
F16 = mybir.dt.float16

# kept for test.py compat; this kernel is fp16-only
GEMM_MODE = "f16"
OUT_MODE = "f16"

_CACHE = {}


def _build(gemm_mode: str = GEMM_MODE, out_mode: str = OUT_MODE, repeat: int = 1):
    out_dt = F32 if out_mode == "f32" else F16
    nc = bacc.Bacc(
        "TRN2",
        target_bir_lowering=False,
        debug=False,
        enable_asserts=False,
        num_devices=N_CORES,
    )

    xT_d = nc.declare_dram_parameter("xT", [IN_F, B_LOC], F16, isOutput=False)
    w1c_d = nc.declare_dram_parameter("w1c", [128, 2 * HA], F16, isOutput=False)
    b1c_d = nc.declare_dram_parameter("b1c", [128, 1], F32, isOutput=False)
    beff_d = nc.declare_dram_parameter("beff", [HA, SEQ], F16, isOutput=False)
    out_d = nc.declare_dram_parameter("out", [B_LOC, SEQ], out_dt, isOutput=True)

    NJ = 2                  # L1 batch chunks of 512
    MM = B_LOC // 128       # 8 batch blocks for the GEMM
    SC = SEQ // 512         # 8 seq chunks

    relu = mybir.ActivationFunctionType.Relu
    copyf = mybir.ActivationFunctionType.Copy

    with tile.TileContext(nc) as tc:
        with (
            tc.tile_pool(name="const", bufs=1) as constp,
            tc.tile_pool(name="data", bufs=1) as datap,
            tc.tile_pool(name="outsb", bufs=3) as outsbp,
            tc.tile_pool(name="h_ps", bufs=2, space="PSUM") as hpp,
            tc.tile_pool(name="o_ps", bufs=3, space="PSUM") as opp,
        ):
            def body():
                w1 = constp.tile([128, 2, HA], F16, tag="w1")
                b1sb = constp.tile([128, 1], F32, tag="b1")
                xsb = datap.tile([128, 2, B_LOC], F16, tag="x")
                beff = datap.tile([HA, SEQ], F16, tag="beff")
                hT = datap.tile([HA, B_LOC], F16, tag="hT")

                xT_pkn = xT_d.rearrange("(k p) n -> p k n", p=128)
                w1_pk = w1c_d.rearrange("p (k m) -> p k m", k=2)

                # consts on the ACT queue (no deps, issued while ACT is
                # idle); x + beff on SP in consumption order.  Data loads
                # must NOT share the ACT queue: a beff load waits on the
                # prior iteration's GEMM reads, and the in-order queue would
                # stall every drain queued behind it.
                nc.scalar.dma_start(w1[:], w1_pk[:])
                nc.scalar.dma_start(b1sb[:], b1c_d[:])
                nc.sync.dma_start(xsb[:, :, 0:512], xT_pkn[:, :, 0:512])
                nc.sync.dma_start(beff[:, 0:1024], beff_d[:, 0:1024])
                nc.sync.dma_start(xsb[:, :, 512:1024], xT_pkn[:, :, 512:1024])
                nc.sync.dma_start(beff[:, 1024:4096], beff_d[:, 1024:4096])

                # L1: hT[65, 1024] = relu(W1aug @ xT + b1aug), row 64 == 1.0
                for j in range(NJ):
                    hp = hpp.tile([HA, 512], F32, tag="h")
                    for k in range(2):
                        nc.tensor.matmul(
                            hp[:],
                            w1[:, k, :],
                            xsb[:, k, ts(j, 512)],
                            start=(k == 0),
                            stop=(k == 1),
                        )
                    nc.scalar.activation(
                        hT[:, ts(j, 512)], hp[:], relu, bias=b1sb[:HA, :]
                    )

                # GEMM: out[mm*128:+128, :] = hT[:, mm-block].T @ beff
                # PSUM drains in 2-bank [128, 1024] chunks, ACT/DVE weighted
                # ~17:15 (ACT is 1.2 GHz vs DVE 0.96, but also does the relus)
                NH = SEQ // 1024        # 4 drain chunks per mm block
                drain_ctr = 0
                for mm in range(MM):
                    osb = outsbp.tile([128, SEQ], out_dt, tag="osb")
                    for sh in range(NH):
                        op = opp.tile([128, 1024], F32, tag="op")
                        for t in range(2):
                            nc.tensor.matmul(
                                op[:, ts(t, 512)],
                                hT[:, ts(mm, 128)],
                                beff[:, ts(2 * sh + t, 512)],
                                start=True,
                                stop=True,
                            )
                        use_act = (drain_ctr * 15) // 32 != ((drain_ctr + 1) * 15) // 32
                        drain_ctr += 1
                        if use_act:
                            nc.scalar.activation(osb[:, ts(sh, 1024)], op[:], copyf)
                        else:
                            nc.vector.tensor_copy(osb[:, ts(sh, 1024)], op[:])
                        if sh == 0 and mm % 3 == 0:
                            # early partial store primes the pipe (SP mms
                            # only: SWDGE's ~1us per-DMA gen cost on the
                            # Pool sequencer makes extra gpsimd DMAs dear)
                            nc.sync.dma_start(
                                out_d[ts(mm, 128), 0:1024], osb[:, 0:1024]
                            )
                    if mm % 3 == 0:
                        nc.sync.dma_start(
                            out_d[ts(mm, 128), 1024:4096], osb[:, 1024:4096]
                        )
                    elif mm % 3 == 1:
                        nc.gpsimd.dma_start(out_d[ts(mm, 128), :], osb[:])
                    else:
                        nc.scalar.dma_start(out_d[ts(mm, 128), :], osb[:])

            if repeat == 1:
                body()
            else:
                with tc.For_i(0, repeat, 1):
                    body()

    nc.compile()
    return nc


def _get_nc(gemm_mode: str = GEMM_MODE, out_mode: str = OUT_MODE, repeat: int = 1):
    key = (gemm_mode, out_mode, repeat)
    if key not in _CACHE:
        _CACHE[key] = _build(gemm_mode, out_mode, repeat)
    return _CACHE[key]


def _fold(W1, b1, Wd1, bd1, Wd2, bd2, Wd3, bd3, W2, b2, bases):
    f8 = np.float64
    W2eff = W2.astype(f8) @ Wd1.astype(f8) @ Wd2.astype(f8) @ Wd3.astype(f8)
    b2eff = b2.astype(f8) + (
        bd3.astype(f8) @ Wd2.astype(f8).T @ Wd1.astype(f8).T
        + bd2.astype(f8) @ Wd1.astype(f8).T
        + bd1.astype(f8)
    ) @ W2.astype(f8).T
    beff = np.empty((HA, SEQ), np.float16)
    beff[:HID] = (W2eff.T @ bases.astype(f8)).astype(np.float16)
    beff[HID] = (b2eff @ bases.astype(f8)).astype(np.float16)

    # W1aug: 65th output channel with zero weights + bias 1 -> relu==1.0
    w1c = np.zeros((128, 2 * HA), np.float16)
    W1T = W1.T.astype(np.float16)          # [256, 64]
    w1c[:, 0:HID] = W1T[:128]
    w1c[:, HA : HA + HID] = W1T[128:]
    b1c = np.zeros((128, 1), np.float32)
    b1c[:HID, 0] = b1
    b1c[HID, 0] = 1.0
    return w1c, b1c, beff


def _in_maps(x, W1, b1, Wd1, bd1, Wd2, bd2, Wd3, bd3, W2, b2, bases,
             gemm_mode=GEMM_MODE):
    w1c, b1c, beff = _fold(W1, b1, Wd1, bd1, Wd2, bd2, Wd3, bd3, W2, b2, bases)
    common = {"w1c": w1c, "b1c": b1c, "beff": beff}
    maps = []
    for i in range(N_CORES):
        m = dict(common)
        m["xT"] = np.ascontiguousarray(
            x[i * B_LOC : (i + 1) * B_LOC].T.astype(np.float16)
        )
        maps.append(m)
    return maps


def run(inputs: dict, gemm_mode: str = GEMM_MODE, out_mode: str = OUT_MODE,
        repeat: int = 1, **run_kwargs):
    """Shard, execute on 8 cores, gather. Returns (out, BassKernelResults)."""
    nc = _get_nc(gemm_mode, out_mode, repeat)
    in_maps = _in_maps(**{k: np.asarray(v) for k, v in inputs.items()},
                       gemm_mode=gemm_mode)
    res = run_bass_kernel_spmd(nc, in_maps, list(range(N_CORES)), **run_kwargs)
    shards = [np.asarray(res.results[i]["out"], dtype=np.float32)
              for i in range(N_CORES)]
    out = np.concatenate(shards, axis=0)
    return out, res


def kernel(**inputs) -> np.ndarray:
    out, _ = run(inputs)
    return out
